# revision 29
# baseline (speedup 1.0000x reference)
"""CPModule (3-axis line-interp product) TRN2 kernel, transfer-optimized.

out[c, n] = prod_a lerp(param_a[c, :], pos_a(n)),  pos = (x+1)*149.5.

Device algorithm (no host-side sorting): per-axis linear interpolation is a
K=128 matmul with a "two-hot" hat-basis matrix e[g, t] = relu(1 - |pos_t - g|).
Grid 300 is split into 3 chunks of 128 lanes at stride 127; ALL three chunks
are computed for every point and accumulated in PSUM (the hat basis is zero
outside the containing chunk; duplicated boundary rows g=127 / g=254 are
zeroed in one of the two tables so each grid row contributes exactly once).

The dominant cost of this problem in this environment is the axon tunnel
(~55-80 MB/s each way, full-duplex), so the kernel minimizes bytes and
overlaps directions:
  - output is quantized on-device to int8 with a per-partition-row scale
    (q = out * 126.5/rowmax, |err| <= rowmax/126.5 < 1% of absmax << 2e-2);
    the f32 scales are bitcast into 4 extra int8 columns of the output
  - the f32->int8 second pass runs in the same program via a DRAM scratch
    tile (rowmax must be final before quantizing)
  - the exec path is a cached jax.jit(shard_map) around _bass_exec_p with
    output backing buffers created device-side and recycled via donation,
    so a warm call uploads only coords (24 MB) + tables (0.3 MB) and
    downloads int8 output (96 MB)
  - the call is split into S pipelined segment launches: segment s+1's
    upload/exec overlaps segment s's download (tunnel is full-duplex), and
    dequantization runs outside the fetch threads.

8 NeuronCores data-parallel over points; tables replicated.
"""

import os
import sys

sys.path.insert(0, "/opt/trn_rl_repo")
os.environ.setdefault("JAX_PLATFORMS", "axon,cpu")

import contextlib
import math
from concurrent.futures import ThreadPoolExecutor

import numpy as np

try:  # keep medium-sized host buffers on the heap so pages get reused
    import ctypes

    ctypes.CDLL("libc.so.6").mallopt(-3, 256 * 1024 * 1024)  # M_MMAP_THRESHOLD
except Exception:
    pass

import concourse.bass as bass
import concourse.mybir as mybir
from concourse import tile

F32 = mybir.dt.float32
I8 = mybir.dt.int8
AF = mybir.ActivationFunctionType
ALU = mybir.AluOpType

N_COMP = 48
G = 300
N_CORES = 8
TILE = 512
GROUP = 2 * TILE  # 1024 points per device group
SLAB = 8  # groups of coords per load slab
QCAP = 126.5  # quantization target range (<127 so saturation can't wrap)
PCHUNK = 4096  # pass-2 scratch columns per tile (multiple of TILE)
U16 = mybir.dt.uint16
POS_SCALE = 299.0 / 65535.0  # u16 fixed-point coord decode: pos = u * POS_SCALE


def _legalize_sync_waits(nc, max_waits=1):
    """This walrus build accepts at most one sync-wait per instruction; split
    extra waits onto preceding same-engine drains (same-queue => in order)."""
    n = 0
    for f in nc.m.functions:
        for bb in f.blocks:
            new_list = []
            for ins in bb.instructions:
                si = ins.sync_info
                waits = list(si.on_wait) if si and si.on_wait else []
                if len(waits) > max_waits:
                    head, tail = waits[:-max_waits], waits[-max_waits:]
                    for w in head:
                        n += 1
                        import bass_rust as _br
                        new_list.append(
                            _br.InstNoOp(
                                name=f"{ins.name}-wsplit-{n}",
                                engine=ins.engine,
                                ins=[],
                                outs=[],
                                sync_info=mybir.SyncInfo(on_wait=[w], on_update=[]),
                            )
                        )
                    ins.sync_info = mybir.SyncInfo(
                        on_wait=tail,
                        on_update=list(si.on_update) if si.on_update else [],
                    )
                new_list.append(ins)
            bb.instructions[:] = new_list
    return n


def _build_program(n_groups, num_devices=N_CORES):
    """Two-pass SPMD program for n_groups*GROUP points per core.

    Output tensor is [48, n_groups*GROUP + 4] int8: quantized values followed
    by 4 columns holding the bitcast f32 quantization multiplier per row.
    """
    npcp = n_groups * GROUP
    scratch_cols = n_groups * TILE  # packed halves: [128, 512] per group

    nc = bass.Bass("TRN2", target_bir_lowering=False, debug=False, num_devices=num_devices)
    d_coords = nc.dram_tensor("coords", [3, npcp], U16, kind="ExternalInput")
    d_lhsT = nc.dram_tensor("lhsT", [9, 128, 64], F32, kind="ExternalInput")
    d_bias = nc.dram_tensor("bias", [128, 3], F32, kind="ExternalInput")
    d_out = nc.dram_tensor("out_q", [N_COMP, npcp + 4], I8, kind="ExternalOutput")

    with tile.TileContext(nc) as tc:
        with contextlib.ExitStack() as ctx:
            const = ctx.enter_context(tc.tile_pool(name="const", bufs=1))
            slabp = ctx.enter_context(tc.tile_pool(name="slabp", bufs=2))
            work = ctx.enter_context(tc.tile_pool(name="work", bufs=2))
            outp = ctx.enter_context(tc.tile_pool(name="outp", bufs=3))
            q2p = ctx.enter_context(tc.tile_pool(name="q2p", bufs=2))
            bcp = ctx.enter_context(tc.tile_pool(name="bcp", bufs=1, space="PSUM"))
            vpp = ctx.enter_context(tc.tile_pool(name="vpp", bufs=6, space="PSUM"))
            dramp = ctx.enter_context(tc.tile_pool(name="dramp", bufs=1, space="DRAM"))

            scratch = dramp.tile([128, scratch_cols], F32)

            lhsT = const.tile([128, 9 * 64], F32)
            nc.sync.dma_start(
                lhsT[:].rearrange("p (n d) -> p n d", d=64),
                d_lhsT.ap().rearrange("n p d -> p n d"),
            )
            biast = const.tile([128, 3], F32)
            nc.sync.dma_start(biast[:], d_bias.ap())
            onest = const.tile([65, 128], F32)
            for a in range(3):
                nc.vector.memset(onest[32 * a : 32 * a + 1, :], 1.0)
            m = const.tile([128, 1], F32)
            nc.vector.memset(m[:], 1e-20)

            # ---- pass 1: interpolate, product, rowmax, f32 scratch ----
            for g in range(n_groups):
                s = g % SLAB
                if s == 0:
                    npts = min(SLAB * GROUP, npcp - g * GROUP)
                    slab_u = slabp.tile([65, SLAB * GROUP], U16, name="slab_u", tag="slab_u")
                    slab = slabp.tile([65, SLAB * GROUP], F32, name="slab", tag="slab")
                    for a in range(3):
                        nc.sync.dma_start(
                            slab_u[32 * a : 32 * a + 1, 0:npts],
                            d_coords.ap()[a : a + 1, g * GROUP : g * GROUP + npts],
                        )
                        nc.vector.tensor_copy(
                            slab[32 * a : 32 * a + 1, 0:npts],
                            slab_u[32 * a : 32 * a + 1, 0:npts],
                        )
                vps = []
                for a in range(3):
                    crow = slab[32 * a : 32 * a + 1, s * GROUP : (s + 1) * GROUP]
                    bc = bcp.tile([128, GROUP], F32, name=f"bc_{g}_{a}", tag="bc")
                    nc.tensor.matmul(
                        bc[:, 0:TILE], onest[32 * a : 32 * a + 1, :], crow[:, 0:TILE],
                        start=True, stop=True,
                    )
                    nc.tensor.matmul(
                        bc[:, TILE:GROUP], onest[32 * a : 32 * a + 1, :], crow[:, TILE:GROUP],
                        start=True, stop=True,
                    )
                    vp = vpp.tile([128, TILE], F32, name=f"vp_{g}_{a}", tag="vp")
                    for c in range(3):
                        tabs = work.tile([128, GROUP], F32, name=f"tabs_{g}_{a}_{c}", tag="tabs", bufs=3)
                        nc.scalar.activation(
                            tabs[:], bc[:], AF.Abs, bias=biast[:, c : c + 1], scale=POS_SCALE
                        )
                        eneg = work.tile([128, GROUP], F32, name=f"eneg_{g}_{a}_{c}", tag="eneg", bufs=3)
                        nc.vector.tensor_scalar(eneg[:], tabs[:], 1.0, 1.0, ALU.min, ALU.subtract)
                        lt = lhsT[:, (a * 3 + c) * 64 : (a * 3 + c + 1) * 64]
                        nc.tensor.matmul(
                            vp[0:64, :], lt, eneg[:, 0:TILE],
                            start=(c == 0), stop=(c == 2), tile_position=(0, 0),
                        )
                        nc.tensor.matmul(
                            vp[64:128, :], lt, eneg[:, TILE:GROUP],
                            start=(c == 0), stop=(c == 2), tile_position=(0, 64),
                        )
                    vps.append(vp)

                v1sb = outp.tile([128, TILE], F32, name=f"v1sb_{g}", tag="v1sb")
                nc.vector.tensor_copy(v1sb[:], vps[1][:])
                p01 = outp.tile([128, TILE], F32, name=f"p01_{g}", tag="p01")
                nc.vector.tensor_mul(p01[:], vps[0][:], v1sb[:])
                outt = outp.tile([128, TILE], F32, name=f"outt_{g}", tag="outt")
                nc.vector.tensor_mul(outt[:], vps[2][:], p01[:])

                mt = outp.tile([128, 1], F32, name=f"mt_{g}", tag="mt")
                nc.vector.tensor_reduce(
                    mt[:], outt[:], axis=mybir.AxisListType.X, op=ALU.max,
                    apply_absolute_value=True,
                )
                nc.vector.tensor_tensor(m[:], m[:], mt[:], op=ALU.max)

                nc.sync.dma_start(scratch[:, g * TILE : (g + 1) * TILE], outt[:])

            # ---- scales: rs = QCAP / max(row, row+64); bitcast into out ----
            mc = const.tile([128, 1], F32)
            nc.vector.memset(mc[:], 1e-20)
            nc.sync.dma_start(mc[0:N_COMP, :], m[64 : 64 + N_COMP, :])
            m2 = const.tile([128, 1], F32)
            nc.vector.memset(m2[:], 1.0)
            nc.vector.tensor_tensor(m2[0:N_COMP, :], m[0:N_COMP, :], mc[0:N_COMP, :], op=ALU.max)
            rs = const.tile([128, 1], F32)
            nc.vector.memset(rs[:], 1.0)
            rinv = const.tile([128, 1], F32)
            nc.vector.memset(rinv[:], 1.0)
            nc.vector.reciprocal(rinv[0:N_COMP, :], m2[0:N_COMP, :])
            nc.vector.tensor_scalar_mul(rs[0:N_COMP, :], rinv[0:N_COMP, :], QCAP)
            nc.sync.dma_start(rs[64 : 64 + N_COMP, :], rs[0:N_COMP, :])
            nc.sync.dma_start(
                d_out.ap()[:, npcp : npcp + 4], rs[0:N_COMP, :].bitcast(I8)
            )

            # ---- pass 2: quantize scratch -> int8 natural layout ----
            out_r = d_out.ap()[:, 0:npcp].rearrange("p (g h j) -> p g h j", h=2, j=TILE)
            n_chunks = math.ceil(scratch_cols / PCHUNK)
            for s in range(n_chunks):
                u0 = s * PCHUNK
                cols = min(PCHUNK, scratch_cols - u0)
                ngr = cols // TILE
                g0 = u0 // TILE
                qa = q2p.tile([128, PCHUNK], F32, name=f"qa_{s}", tag="qa")
                nc.sync.dma_start(qa[:, 0:cols], scratch[:, u0 : u0 + cols])
                qi = q2p.tile([128, PCHUNK], I8, name=f"qi_{s}", tag="qi")
                nc.vector.tensor_scalar_mul(qi[:, 0:cols], qa[:, 0:cols], rs[:, 0:1])
                nc.sync.dma_start(
                    out_r[:, g0 : g0 + ngr, 0, :],
                    qi[0:N_COMP, 0:cols].rearrange("p (g j) -> p g j", j=TILE),
                )
                nc.sync.dma_start(
                    out_r[:, g0 : g0 + ngr, 1, :],
                    qi[64 : 64 + N_COMP, 0:cols].rearrange("p (g j) -> p g j", j=TILE),
                )

    from concourse.hw_specs import get_activation_tables
    import bass_rust as _br
    _br.insert_act_table_loads(nc, list(get_activation_tables(nc.m.arch).items()))
    _legalize_sync_waits(nc)
    return nc


# ---------------------------------------------------------------------------
# Cached PJRT exec path (modeled on concourse.bass2jax.run_bass_via_pjrt, but
# with a persistent jitted executable and donated, device-recycled output
# backing buffers so warm calls transfer no output-sized zeros).
# ---------------------------------------------------------------------------

_EXEC_CACHE: dict = {}
_HOST_BUFS: dict = {}  # n -> (out f32 [48,n], enc scratch f32 [3,n], u16 [3,n])


def _get_exec(seg_groups):
    key = seg_groups
    if key in _EXEC_CACHE:
        return _EXEC_CACHE[key]

    import jax
    import jax.numpy as jnp
    from jax.sharding import Mesh, PartitionSpec, NamedSharding
    try:
        from jax.experimental.shard_map import shard_map
    except ImportError:
        from jax.sharding import shard_map  # newer jax
    from concourse import bass2jax

    bass2jax.install_neuronx_cc_hook()

    nc = _build_program(seg_groups)
    partition_name = nc.partition_id_tensor.name if nc.partition_id_tensor else None

    in_names, out_names, out_avals = [], [], []
    for alloc in nc.m.functions[0].allocations:
        if not isinstance(alloc, mybir.MemoryLocationSet):
            continue
        name = alloc.memorylocations[0].name
        if alloc.kind == "ExternalInput":
            if name != partition_name:
                in_names.append(name)
        elif alloc.kind == "ExternalOutput":
            shape = tuple(alloc.tensor_shape)
            dtype = mybir.dt.np(alloc.dtype)
            out_names.append(name)
            out_avals.append(jax.core.ShapedArray(shape, dtype))
    n_params = len(in_names)
    n_outs = len(out_names)
    in_names = in_names + out_names
    if partition_name is not None:
        in_names.append(partition_name)

    dbg_names = []
    if nc.dbg_addr is not None:
        assert not nc.dbg_callbacks
        dbg_names = [nc.dbg_addr.name]

    def _body(*args):
        operands = list(args)
        if partition_name is not None:
            operands.append(bass2jax.partition_id_tensor())
        outs = bass2jax._bass_exec_p.bind(
            *operands,
            out_avals=tuple(out_avals),
            in_names=tuple(in_names),
            out_names=tuple(out_names),
            lowering_input_output_aliases=(),
            sim_require_finite=True,
            sim_require_nnan=True,
            nc=nc,
        )
        return tuple(outs)

    devices = jax.devices()[:N_CORES]
    assert len(devices) == N_CORES
    mesh = Mesh(np.asarray(devices), ("core",))
    sharding = NamedSharding(mesh, PartitionSpec("core"))
    repl_sharding = NamedSharding(mesh, PartitionSpec())
    # coords is per-core data; tables are replicated; backings are per-core
    replicated = {"lhsT", "bias"}
    in_specs = tuple(
        PartitionSpec() if nm.split("/")[-1] in replicated else PartitionSpec("core")
        for nm in in_names[:n_params]
    ) + (PartitionSpec("core"),) * n_outs
    out_specs = (PartitionSpec("core"),) * n_outs
    donate = tuple(range(n_params, n_params + n_outs))
    fn = jax.jit(
        shard_map(_body, mesh=mesh, in_specs=in_specs, out_specs=out_specs, check_rep=False),
        donate_argnums=donate,
        keep_unused=True,
    )

    init_shapes = [
        (tuple([N_CORES * av.shape[0]] + list(av.shape[1:])), av.dtype) for av in out_avals
    ]
    init = jax.jit(
        lambda: tuple(jnp.zeros(s, d) for s, d in init_shapes),
        out_shardings=tuple(sharding for _ in init_shapes),
    )

    state = {
        "fn": fn,
        "init": init,
        "in_names": in_names[:n_params],
        "out_names": out_names,
        "backings": {},  # seg index -> tuple of backing arrays
        "sharding": sharding,
        "repl_sharding": repl_sharding,
        "devices": devices,
        "dbg_names": dbg_names,
    }
    _EXEC_CACHE[key] = state
    return state


def _schedule(n_groups):
    """Per-segment group counts: small first segments so download bytes start
    flowing early, then large segments for streaming efficiency."""
    env = os.environ.get("KSCHED")
    if env:
        sched = [int(x) for x in env.split(",")]
        assert sum(sched) == n_groups, (sched, n_groups)
        return sched
    if n_groups <= 12:
        return [n_groups]
    sched = []
    size, left = 8, n_groups
    while left > 2 * size and size < 128:
        sched.append(size)
        left -= size
        size *= 2
    while left > 0:
        take = min(128, left)
        sched.append(take)
        left -= take
    return sched


def kernel(xyz_sampled, param0, param1, param2):
    import jax
    import time as _time

    prof = bool(int(os.environ.get("KPROF", "0")))
    _t0 = _time.perf_counter()

    xyz = np.ascontiguousarray(xyz_sampled, dtype=np.float32)
    params = [
        np.ascontiguousarray(p.reshape(p.shape[1], p.shape[2]), dtype=np.float32)
        for p in (param0, param1, param2)
    ]
    n = xyz.shape[0]
    assert n % N_CORES == 0
    npc = n // N_CORES
    n_groups = math.ceil(npc / GROUP)
    sched = _schedule(n_groups)
    S = len(sched)

    sts = []  # (state, backing instance index) per segment
    seen: dict = {}
    for sg in sched:
        st = _get_exec(sg)
        inst = seen.get(sg, 0)
        seen[sg] = inst + 1
        if inst not in st["backings"]:
            st["backings"][inst] = list(st["init"]())
        sts.append((st, inst))
    st0 = sts[0][0]
    devices = st0["devices"]
    sharding = st0["sharding"]

    # --- host prep: u16 fixed-point coords; tables from params ---
    # u = floor((x+1)*32767.5 + 0.5), pos = u * (299/65535); |pos err| <= 0.00228
    if n not in _HOST_BUFS:
        _HOST_BUFS[n] = (
            np.empty((N_COMP, n), dtype=np.float32),
            np.empty((3, n), dtype=np.float32),
        )
    out, scr = _HOST_BUFS[n]
    np.multiply(xyz.T, 32767.5, out=scr)
    np.add(scr, 32768.0, out=scr)  # +32767.5 offset +0.5 for round-on-truncate
    xyzT = scr.astype(np.uint16)  # [3, n]
    lhsT9 = np.zeros((9, 128, 64), dtype=np.float32)
    for a in range(3):
        for c in range(3):
            seg = params[a][:, 127 * c : min(127 * c + 128, G)]
            lhsT9[a * 3 + c, : seg.shape[1], :N_COMP] = -seg.T
        lhsT9[a * 3 + 0, 127, :] = 0.0  # g=127 kept in chunk1 lane 0
        lhsT9[a * 3 + 1, 127, :] = 0.0  # g=254 kept in chunk2 lane 0
    bias = np.zeros((128, 3), dtype=np.float32)
    for c in range(3):
        bias[:, c] = -(127.0 * c + np.arange(128))

    lhsT_g = jax.device_put(lhsT9, st0["repl_sharding"])
    bias_g = jax.device_put(bias, st0["repl_sharding"])
    dbg_g = None
    if st0["dbg_names"]:
        dbg_g = jax.device_put(np.zeros((N_CORES, 2), np.uint32), sharding)

    # --- dispatch all segments (async) and queue D2H copies immediately;
    # the tunnel is full-duplex so segment s's download streams while
    # segment s+1 uploads/executes ---
    tasks = []
    g0 = 0
    for s in range(S):
        st, inst = sts[s]
        seg_npcp = sched[s] * GROUP
        c0 = g0 * GROUP
        cg = np.empty((N_CORES * 3, seg_npcp), dtype=np.uint16)
        cols = min(seg_npcp, npc - c0)  # last segment: real cols before padding
        for k in range(N_CORES):
            lo = k * npc + c0
            cg[3 * k : 3 * k + 3, :cols] = xyzT[:, lo : lo + cols]
            if cols < seg_npcp:
                cg[3 * k : 3 * k + 3, cols:] = cg[3 * k : 3 * k + 3, cols - 1 : cols]
        coords_g = jax.device_put(cg, sharding)
        args = []
        for name in st["in_names"]:
            base = name.split("/")[-1]
            if base == "coords":
                args.append(coords_g)
            elif base == "lhsT":
                args.append(lhsT_g)
            elif base == "bias":
                args.append(bias_g)
            elif st["dbg_names"] and base == st["dbg_names"][0]:
                args.append(dbg_g)
            else:
                raise KeyError(f"unexpected program input {name}")
        outs = st["fn"](*args, *st["backings"][inst])
        st["backings"][inst] = list(outs)
        oq = outs[st["out_names"].index("out_q")]
        for sh in sorted(oq.addressable_shards, key=lambda x: x.index[0].start):
            sh.data.copy_to_host_async()
            tasks.append((c0, seg_npcp, sh.index[0].start // N_COMP, sh.data))
        g0 += sched[s]
    if prof:
        print(f"[kprof] {_time.perf_counter()-_t0:.3f} dispatched+copies queued")

    # --- collect (main thread waits on streaming copies); dequant in pool ---
    def dequant(c0, seg_npcp, k, qk):
        cols = min(seg_npcp, npc - c0)
        scale = (
            1.0 / qk[:, seg_npcp : seg_npcp + 4].copy().view(np.float32).astype(np.float64)
        ).astype(np.float32)
        dst = out[:, k * npc + c0 : k * npc + c0 + cols]
        np.multiply(qk[:, :cols], scale, out=dst, dtype=np.float32)

    t_wait = 0.0
    marks = []
    with ThreadPoolExecutor(max_workers=2) as dpool:
        futs = []
        for c0, seg_npcp, k, shard in tasks:
            _tw = _time.perf_counter()
            qk = np.asarray(shard)
            t_wait += _time.perf_counter() - _tw
            if prof:
                marks.append(_time.perf_counter() - _t0)
            futs.append(dpool.submit(dequant, c0, seg_npcp, k, qk))
        for f in futs:
            f.result()
    if prof:
        nm = len(marks)
        print(
            f"[kprof] {_time.perf_counter()-_t0:.3f} done; wait {t_wait:.3f}; "
            f"task done t: first {marks[0]:.3f} q1 {marks[nm//4]:.3f} "
            f"med {marks[nm//2]:.3f} q3 {marks[3*nm//4]:.3f} last {marks[-1]:.3f}"
        )
    return out


if __name__ == "__main__":
    rng = np.random.default_rng(0)
    n = int(os.environ.get("KN", 16 * 1024))
    xyz = rng.uniform(-1, 1, size=(n, 3)).astype(np.float32)
    ps = [0.2 * rng.standard_normal((1, N_COMP, G, 1)).astype(np.float32) for _ in range(3)]

    def ref_interp(p, coord):
        pp = p[0, :, :, 0]
        pos = (coord + 1.0) * 0.5 * (G - 1)
        i0 = np.clip(np.floor(pos).astype(np.int64), 0, G - 1)
        i1 = np.minimum(i0 + 1, G - 1)
        w = (pos - i0).astype(np.float32)
        return pp[:, i0] * (1.0 - w) + pp[:, i1] * w

    exp = ref_interp(ps[0], xyz[:, 0]) * ref_interp(ps[1], xyz[:, 1]) * ref_interp(ps[2], xyz[:, 2])
    got = kernel(xyz, *ps)
    err = np.abs(got - exp).max()
    print("max abs err:", err, "absmax:", np.abs(exp).max(), "rel:", err / np.abs(exp).max())
    import time
    for _ in range(2):
        t0 = time.perf_counter()
        kernel(xyz, *ps)
        print("warm wall:", time.perf_counter() - t0)


# revision 30
# speedup vs baseline: 1.0277x; 1.0277x over previous
"""CPModule (3-axis line-interp product) TRN2 kernel, transfer-optimized.

out[c, n] = prod_a lerp(param_a[c, :], pos_a(n)),  pos = (x+1)*149.5.

Device algorithm (no host-side sorting): per-axis linear interpolation is a
K=128 matmul with a "two-hot" hat-basis matrix e[g, t] = relu(1 - |pos_t - g|).
Grid 300 is split into 3 chunks of 128 lanes at stride 127; ALL three chunks
are computed for every point and accumulated in PSUM (the hat basis is zero
outside the containing chunk; duplicated boundary rows g=127 / g=254 are
zeroed in one of the two tables so each grid row contributes exactly once).

The dominant cost of this problem in this environment is the axon tunnel
(~55-80 MB/s each way, full-duplex), so the kernel minimizes bytes and
overlaps directions:
  - output is quantized on-device to int8 with a per-partition-row scale
    (q = out * 126.5/rowmax, |err| <= rowmax/126.5 < 1% of absmax << 2e-2);
    the f32 scales are bitcast into 4 extra int8 columns of the output
  - the f32->int8 second pass runs in the same program via a DRAM scratch
    tile (rowmax must be final before quantizing)
  - the exec path is a cached jax.jit(shard_map) around _bass_exec_p with
    output backing buffers created device-side and recycled via donation,
    so a warm call uploads only coords (24 MB) + tables (0.3 MB) and
    downloads int8 output (96 MB)
  - the call is split into S pipelined segment launches: segment s+1's
    upload/exec overlaps segment s's download (tunnel is full-duplex), and
    dequantization runs outside the fetch threads.

8 NeuronCores data-parallel over points; tables replicated.
"""

import os
import sys

sys.path.insert(0, "/opt/trn_rl_repo")
os.environ.setdefault("JAX_PLATFORMS", "axon,cpu")

import contextlib
import math
from concurrent.futures import ThreadPoolExecutor

import numpy as np

try:  # keep medium-sized host buffers on the heap so pages get reused
    import ctypes

    ctypes.CDLL("libc.so.6").mallopt(-3, 256 * 1024 * 1024)  # M_MMAP_THRESHOLD
except Exception:
    pass

import concourse.bass as bass
import concourse.mybir as mybir
from concourse import tile

F32 = mybir.dt.float32
I8 = mybir.dt.int8
AF = mybir.ActivationFunctionType
ALU = mybir.AluOpType

N_COMP = 48
G = 300
N_CORES = 8
TILE = 512
GROUP = 2 * TILE  # 1024 points per device group
SLAB = 8  # groups of coords per load slab
QCAP = 126.5  # quantization target range (<127 so saturation can't wrap)
PCHUNK = 4096  # pass-2 scratch columns per tile (multiple of TILE)
U16 = mybir.dt.uint16
POS_SCALE = 299.0 / 65535.0  # u16 fixed-point coord decode: pos = u * POS_SCALE


def _legalize_sync_waits(nc, max_waits=1):
    """This walrus build accepts at most one sync-wait per instruction; split
    extra waits onto preceding same-engine drains (same-queue => in order)."""
    n = 0
    for f in nc.m.functions:
        for bb in f.blocks:
            new_list = []
            for ins in bb.instructions:
                si = ins.sync_info
                waits = list(si.on_wait) if si and si.on_wait else []
                if len(waits) > max_waits:
                    head, tail = waits[:-max_waits], waits[-max_waits:]
                    for w in head:
                        n += 1
                        import bass_rust as _br
                        new_list.append(
                            _br.InstNoOp(
                                name=f"{ins.name}-wsplit-{n}",
                                engine=ins.engine,
                                ins=[],
                                outs=[],
                                sync_info=mybir.SyncInfo(on_wait=[w], on_update=[]),
                            )
                        )
                    ins.sync_info = mybir.SyncInfo(
                        on_wait=tail,
                        on_update=list(si.on_update) if si.on_update else [],
                    )
                new_list.append(ins)
            bb.instructions[:] = new_list
    return n


def _build_program(n_groups, num_devices=N_CORES):
    """Two-pass SPMD program for n_groups*GROUP points per core.

    Output tensor is [48, n_groups*GROUP + 4] int8: quantized values followed
    by 4 columns holding the bitcast f32 quantization multiplier per row.
    """
    npcp = n_groups * GROUP
    scratch_cols = n_groups * TILE  # packed halves: [128, 512] per group

    nc = bass.Bass("TRN2", target_bir_lowering=False, debug=False, num_devices=num_devices)
    d_coords = nc.dram_tensor("coords", [3, npcp], U16, kind="ExternalInput")
    d_lhsT = nc.dram_tensor("lhsT", [9, 128, 64], F32, kind="ExternalInput")
    d_bias = nc.dram_tensor("bias", [128, 3], F32, kind="ExternalInput")
    d_out = nc.dram_tensor("out_q", [N_COMP, npcp + 4], I8, kind="ExternalOutput")

    with tile.TileContext(nc) as tc:
        with contextlib.ExitStack() as ctx:
            const = ctx.enter_context(tc.tile_pool(name="const", bufs=1))
            slabp = ctx.enter_context(tc.tile_pool(name="slabp", bufs=2))
            work = ctx.enter_context(tc.tile_pool(name="work", bufs=2))
            outp = ctx.enter_context(tc.tile_pool(name="outp", bufs=3))
            q2p = ctx.enter_context(tc.tile_pool(name="q2p", bufs=2))
            bcp = ctx.enter_context(tc.tile_pool(name="bcp", bufs=1, space="PSUM"))
            vpp = ctx.enter_context(tc.tile_pool(name="vpp", bufs=6, space="PSUM"))
            dramp = ctx.enter_context(tc.tile_pool(name="dramp", bufs=1, space="DRAM"))

            scratch = dramp.tile([128, scratch_cols], F32)

            lhsT = const.tile([128, 9 * 64], F32)
            nc.sync.dma_start(
                lhsT[:].rearrange("p (n d) -> p n d", d=64),
                d_lhsT.ap().rearrange("n p d -> p n d"),
            )
            biast = const.tile([128, 3], F32)
            nc.sync.dma_start(biast[:], d_bias.ap())
            onest = const.tile([65, 128], F32)
            for a in range(3):
                nc.vector.memset(onest[32 * a : 32 * a + 1, :], 1.0)
            m = const.tile([128, 1], F32)
            nc.vector.memset(m[:], 1e-20)

            # ---- pass 1: interpolate, product, rowmax, f32 scratch ----
            for g in range(n_groups):
                s = g % SLAB
                if s == 0:
                    npts = min(SLAB * GROUP, npcp - g * GROUP)
                    slab_u = slabp.tile([65, SLAB * GROUP], U16, name="slab_u", tag="slab_u")
                    slab = slabp.tile([65, SLAB * GROUP], F32, name="slab", tag="slab")
                    for a in range(3):
                        nc.sync.dma_start(
                            slab_u[32 * a : 32 * a + 1, 0:npts],
                            d_coords.ap()[a : a + 1, g * GROUP : g * GROUP + npts],
                        )
                        nc.vector.tensor_copy(
                            slab[32 * a : 32 * a + 1, 0:npts],
                            slab_u[32 * a : 32 * a + 1, 0:npts],
                        )
                vps = []
                for a in range(3):
                    crow = slab[32 * a : 32 * a + 1, s * GROUP : (s + 1) * GROUP]
                    bc = bcp.tile([128, GROUP], F32, name=f"bc_{g}_{a}", tag="bc")
                    nc.tensor.matmul(
                        bc[:, 0:TILE], onest[32 * a : 32 * a + 1, :], crow[:, 0:TILE],
                        start=True, stop=True,
                    )
                    nc.tensor.matmul(
                        bc[:, TILE:GROUP], onest[32 * a : 32 * a + 1, :], crow[:, TILE:GROUP],
                        start=True, stop=True,
                    )
                    vp = vpp.tile([128, TILE], F32, name=f"vp_{g}_{a}", tag="vp")
                    for c in range(3):
                        tabs = work.tile([128, GROUP], F32, name=f"tabs_{g}_{a}_{c}", tag="tabs", bufs=3)
                        nc.scalar.activation(
                            tabs[:], bc[:], AF.Abs, bias=biast[:, c : c + 1], scale=POS_SCALE
                        )
                        eneg = work.tile([128, GROUP], F32, name=f"eneg_{g}_{a}_{c}", tag="eneg", bufs=3)
                        nc.vector.tensor_scalar(eneg[:], tabs[:], 1.0, 1.0, ALU.min, ALU.subtract)
                        lt = lhsT[:, (a * 3 + c) * 64 : (a * 3 + c + 1) * 64]
                        nc.tensor.matmul(
                            vp[0:64, :], lt, eneg[:, 0:TILE],
                            start=(c == 0), stop=(c == 2), tile_position=(0, 0),
                        )
                        nc.tensor.matmul(
                            vp[64:128, :], lt, eneg[:, TILE:GROUP],
                            start=(c == 0), stop=(c == 2), tile_position=(0, 64),
                        )
                    vps.append(vp)

                v1sb = outp.tile([128, TILE], F32, name=f"v1sb_{g}", tag="v1sb")
                nc.vector.tensor_copy(v1sb[:], vps[1][:])
                p01 = outp.tile([128, TILE], F32, name=f"p01_{g}", tag="p01")
                nc.vector.tensor_mul(p01[:], vps[0][:], v1sb[:])
                outt = outp.tile([128, TILE], F32, name=f"outt_{g}", tag="outt")
                nc.vector.tensor_mul(outt[:], vps[2][:], p01[:])

                mt = outp.tile([128, 1], F32, name=f"mt_{g}", tag="mt")
                nc.vector.tensor_reduce(
                    mt[:], outt[:], axis=mybir.AxisListType.X, op=ALU.max,
                    apply_absolute_value=True,
                )
                nc.vector.tensor_tensor(m[:], m[:], mt[:], op=ALU.max)

                nc.sync.dma_start(scratch[:, g * TILE : (g + 1) * TILE], outt[:])

            # ---- scales: rs = QCAP / max(row, row+64); bitcast into out ----
            mc = const.tile([128, 1], F32)
            nc.vector.memset(mc[:], 1e-20)
            nc.sync.dma_start(mc[0:N_COMP, :], m[64 : 64 + N_COMP, :])
            m2 = const.tile([128, 1], F32)
            nc.vector.memset(m2[:], 1.0)
            nc.vector.tensor_tensor(m2[0:N_COMP, :], m[0:N_COMP, :], mc[0:N_COMP, :], op=ALU.max)
            rs = const.tile([128, 1], F32)
            nc.vector.memset(rs[:], 1.0)
            rinv = const.tile([128, 1], F32)
            nc.vector.memset(rinv[:], 1.0)
            nc.vector.reciprocal(rinv[0:N_COMP, :], m2[0:N_COMP, :])
            nc.vector.tensor_scalar_mul(rs[0:N_COMP, :], rinv[0:N_COMP, :], QCAP)
            nc.sync.dma_start(rs[64 : 64 + N_COMP, :], rs[0:N_COMP, :])
            nc.sync.dma_start(
                d_out.ap()[:, npcp : npcp + 4], rs[0:N_COMP, :].bitcast(I8)
            )

            # ---- pass 2: quantize scratch -> int8 natural layout ----
            out_r = d_out.ap()[:, 0:npcp].rearrange("p (g h j) -> p g h j", h=2, j=TILE)
            n_chunks = math.ceil(scratch_cols / PCHUNK)
            for s in range(n_chunks):
                u0 = s * PCHUNK
                cols = min(PCHUNK, scratch_cols - u0)
                ngr = cols // TILE
                g0 = u0 // TILE
                qa = q2p.tile([128, PCHUNK], F32, name=f"qa_{s}", tag="qa")
                nc.sync.dma_start(qa[:, 0:cols], scratch[:, u0 : u0 + cols])
                qi = q2p.tile([128, PCHUNK], I8, name=f"qi_{s}", tag="qi")
                nc.vector.tensor_scalar_mul(qi[:, 0:cols], qa[:, 0:cols], rs[:, 0:1])
                nc.sync.dma_start(
                    out_r[:, g0 : g0 + ngr, 0, :],
                    qi[0:N_COMP, 0:cols].rearrange("p (g j) -> p g j", j=TILE),
                )
                nc.sync.dma_start(
                    out_r[:, g0 : g0 + ngr, 1, :],
                    qi[64 : 64 + N_COMP, 0:cols].rearrange("p (g j) -> p g j", j=TILE),
                )

    from concourse.hw_specs import get_activation_tables
    import bass_rust as _br
    _br.insert_act_table_loads(nc, list(get_activation_tables(nc.m.arch).items()))
    _legalize_sync_waits(nc)
    return nc


# ---------------------------------------------------------------------------
# Cached PJRT exec path (modeled on concourse.bass2jax.run_bass_via_pjrt, but
# with a persistent jitted executable and donated, device-recycled output
# backing buffers so warm calls transfer no output-sized zeros).
# ---------------------------------------------------------------------------

_EXEC_CACHE: dict = {}
_HOST_BUFS: dict = {}  # n -> (out f32 [48,n], enc scratch f32 [3,n], u16 [3,n])


def _get_exec(seg_groups):
    key = seg_groups
    if key in _EXEC_CACHE:
        return _EXEC_CACHE[key]

    import jax
    import jax.numpy as jnp
    from jax.sharding import Mesh, PartitionSpec, NamedSharding
    try:
        from jax.experimental.shard_map import shard_map
    except ImportError:
        from jax.sharding import shard_map  # newer jax
    from concourse import bass2jax

    bass2jax.install_neuronx_cc_hook()

    nc = _build_program(seg_groups)
    partition_name = nc.partition_id_tensor.name if nc.partition_id_tensor else None

    in_names, out_names, out_avals = [], [], []
    for alloc in nc.m.functions[0].allocations:
        if not isinstance(alloc, mybir.MemoryLocationSet):
            continue
        name = alloc.memorylocations[0].name
        if alloc.kind == "ExternalInput":
            if name != partition_name:
                in_names.append(name)
        elif alloc.kind == "ExternalOutput":
            shape = tuple(alloc.tensor_shape)
            dtype = mybir.dt.np(alloc.dtype)
            out_names.append(name)
            out_avals.append(jax.core.ShapedArray(shape, dtype))
    n_params = len(in_names)
    n_outs = len(out_names)
    in_names = in_names + out_names
    if partition_name is not None:
        in_names.append(partition_name)

    dbg_names = []
    if nc.dbg_addr is not None:
        assert not nc.dbg_callbacks
        dbg_names = [nc.dbg_addr.name]

    def _body(*args):
        operands = list(args)
        if partition_name is not None:
            operands.append(bass2jax.partition_id_tensor())
        outs = bass2jax._bass_exec_p.bind(
            *operands,
            out_avals=tuple(out_avals),
            in_names=tuple(in_names),
            out_names=tuple(out_names),
            lowering_input_output_aliases=(),
            sim_require_finite=True,
            sim_require_nnan=True,
            nc=nc,
        )
        return tuple(outs)

    devices = jax.devices()[:N_CORES]
    assert len(devices) == N_CORES
    mesh = Mesh(np.asarray(devices), ("core",))
    sharding = NamedSharding(mesh, PartitionSpec("core"))
    repl_sharding = NamedSharding(mesh, PartitionSpec())
    # coords is per-core data; tables are replicated; backings are per-core
    replicated = {"lhsT", "bias"}
    in_specs = tuple(
        PartitionSpec() if nm.split("/")[-1] in replicated else PartitionSpec("core")
        for nm in in_names[:n_params]
    ) + (PartitionSpec("core"),) * n_outs
    out_specs = (PartitionSpec("core"),) * n_outs
    donate = tuple(range(n_params, n_params + n_outs))
    fn = jax.jit(
        shard_map(_body, mesh=mesh, in_specs=in_specs, out_specs=out_specs, check_rep=False),
        donate_argnums=donate,
        keep_unused=True,
    )

    init_shapes = [
        (tuple([N_CORES * av.shape[0]] + list(av.shape[1:])), av.dtype) for av in out_avals
    ]
    init = jax.jit(
        lambda: tuple(jnp.zeros(s, d) for s, d in init_shapes),
        out_shardings=tuple(sharding for _ in init_shapes),
    )

    state = {
        "fn": fn,
        "init": init,
        "in_names": in_names[:n_params],
        "out_names": out_names,
        "backings": {},  # seg index -> tuple of backing arrays
        "sharding": sharding,
        "repl_sharding": repl_sharding,
        "devices": devices,
        "dbg_names": dbg_names,
    }
    _EXEC_CACHE[key] = state
    return state


def _schedule(n_groups):
    """Per-segment group counts: small first segments so download bytes start
    flowing early, then large segments for streaming efficiency."""
    env = os.environ.get("KSCHED")
    if env:
        sched = [int(x) for x in env.split(",")]
        assert sum(sched) == n_groups, (sched, n_groups)
        return sched
    if n_groups <= 12:
        return [n_groups]
    sched = []
    size, left = 8, n_groups
    while left > 2 * size and size < 128:
        sched.append(size)
        left -= size
        size *= 2
    while left > 0:
        take = min(128, left)
        sched.append(take)
        left -= take
    return sched


def kernel(xyz_sampled, param0, param1, param2):
    import jax
    import time as _time

    prof = bool(int(os.environ.get("KPROF", "0")))
    _t0 = _time.perf_counter()

    xyz = np.ascontiguousarray(xyz_sampled, dtype=np.float32)
    params = [
        np.ascontiguousarray(p.reshape(p.shape[1], p.shape[2]), dtype=np.float32)
        for p in (param0, param1, param2)
    ]
    n = xyz.shape[0]
    assert n % N_CORES == 0
    npc = n // N_CORES
    n_groups = math.ceil(npc / GROUP)
    sched = _schedule(n_groups)
    S = len(sched)

    sts = []  # (state, backing instance index) per segment
    seen: dict = {}
    for sg in sched:
        st = _get_exec(sg)
        inst = seen.get(sg, 0)
        seen[sg] = inst + 1
        if inst not in st["backings"]:
            st["backings"][inst] = list(st["init"]())
        sts.append((st, inst))
    st0 = sts[0][0]
    devices = st0["devices"]
    sharding = st0["sharding"]

    # --- host prep: u16 fixed-point coords; tables from params ---
    # u = floor((x+1)*32767.5 + 0.5), pos = u * (299/65535); |pos err| <= 0.00228
    if n not in _HOST_BUFS:
        _HOST_BUFS[n] = (
            np.empty((N_COMP, n), dtype=np.float32),
            np.empty((3, n), dtype=np.float32),
        )
    out, scr = _HOST_BUFS[n]
    np.multiply(xyz.T, 32767.5, out=scr)
    np.add(scr, 32768.0, out=scr)  # +32767.5 offset +0.5 for round-on-truncate
    xyzT = scr.astype(np.uint16)  # [3, n]
    lhsT9 = np.zeros((9, 128, 64), dtype=np.float32)
    for a in range(3):
        for c in range(3):
            seg = params[a][:, 127 * c : min(127 * c + 128, G)]
            lhsT9[a * 3 + c, : seg.shape[1], :N_COMP] = -seg.T
        lhsT9[a * 3 + 0, 127, :] = 0.0  # g=127 kept in chunk1 lane 0
        lhsT9[a * 3 + 1, 127, :] = 0.0  # g=254 kept in chunk2 lane 0
    bias = np.zeros((128, 3), dtype=np.float32)
    for c in range(3):
        bias[:, c] = -(127.0 * c + np.arange(128))

    lhsT_g = jax.device_put(lhsT9, st0["repl_sharding"])
    bias_g = jax.device_put(bias, st0["repl_sharding"])
    dbg_g = None
    if st0["dbg_names"]:
        dbg_g = jax.device_put(np.zeros((N_CORES, 2), np.uint32), sharding)

    # --- dispatch all segments (async) and queue D2H copies immediately;
    # the tunnel is full-duplex so segment s's download streams while
    # segment s+1 uploads/executes ---
    tasks = []
    g0 = 0
    for s in range(S):
        st, inst = sts[s]
        seg_npcp = sched[s] * GROUP
        c0 = g0 * GROUP
        cg = np.empty((N_CORES * 3, seg_npcp), dtype=np.uint16)
        cols = min(seg_npcp, npc - c0)  # last segment: real cols before padding
        for k in range(N_CORES):
            lo = k * npc + c0
            cg[3 * k : 3 * k + 3, :cols] = xyzT[:, lo : lo + cols]
            if cols < seg_npcp:
                cg[3 * k : 3 * k + 3, cols:] = cg[3 * k : 3 * k + 3, cols - 1 : cols]
        coords_g = jax.device_put(cg, sharding)
        args = []
        for name in st["in_names"]:
            base = name.split("/")[-1]
            if base == "coords":
                args.append(coords_g)
            elif base == "lhsT":
                args.append(lhsT_g)
            elif base == "bias":
                args.append(bias_g)
            elif st["dbg_names"] and base == st["dbg_names"][0]:
                args.append(dbg_g)
            else:
                raise KeyError(f"unexpected program input {name}")
        outs = st["fn"](*args, *st["backings"][inst])
        st["backings"][inst] = list(outs)
        oq = outs[st["out_names"].index("out_q")]
        for sh in sorted(oq.addressable_shards, key=lambda x: x.index[0].start):
            sh.data.copy_to_host_async()
            tasks.append((c0, seg_npcp, sh.index[0].start // N_COMP, sh.data))
        g0 += sched[s]
    if prof:
        print(f"[kprof] {_time.perf_counter()-_t0:.3f} dispatched+copies queued")

    # --- collect all shards first (any Python work during streaming stalls
    # the axon receiver threads), then dequantize ---
    qks = []
    for c0, seg_npcp, k, shard in tasks:
        qks.append((c0, seg_npcp, k, np.asarray(shard)))
    if prof:
        print(f"[kprof] {_time.perf_counter()-_t0:.3f} collected")

    for c0, seg_npcp, k, qk in qks:
        cols = min(seg_npcp, npc - c0)
        scale = (
            1.0 / qk[:, seg_npcp : seg_npcp + 4].copy().view(np.float32).astype(np.float64)
        ).astype(np.float32)
        dst = out[:, k * npc + c0 : k * npc + c0 + cols]
        np.multiply(qk[:, :cols], scale, out=dst, dtype=np.float32)
    if prof:
        print(f"[kprof] {_time.perf_counter()-_t0:.3f} done")
    return out


if __name__ == "__main__":
    rng = np.random.default_rng(0)
    n = int(os.environ.get("KN", 16 * 1024))
    xyz = rng.uniform(-1, 1, size=(n, 3)).astype(np.float32)
    ps = [0.2 * rng.standard_normal((1, N_COMP, G, 1)).astype(np.float32) for _ in range(3)]

    def ref_interp(p, coord):
        pp = p[0, :, :, 0]
        pos = (coord + 1.0) * 0.5 * (G - 1)
        i0 = np.clip(np.floor(pos).astype(np.int64), 0, G - 1)
        i1 = np.minimum(i0 + 1, G - 1)
        w = (pos - i0).astype(np.float32)
        return pp[:, i0] * (1.0 - w) + pp[:, i1] * w

    exp = ref_interp(ps[0], xyz[:, 0]) * ref_interp(ps[1], xyz[:, 1]) * ref_interp(ps[2], xyz[:, 2])
    got = kernel(xyz, *ps)
    err = np.abs(got - exp).max()
    print("max abs err:", err, "absmax:", np.abs(exp).max(), "rel:", err / np.abs(exp).max())
    import time
    for _ in range(2):
        t0 = time.perf_counter()
        kernel(xyz, *ps)
        print("warm wall:", time.perf_counter() - t0)


# revision 39
# speedup vs baseline: 1.2847x; 1.2501x over previous
"""CPModule (3-axis line-interp product) TRN2 kernel, transfer-optimized.

out[c, n] = prod_a lerp(param_a[c, :], pos_a(n)),  pos = (x+1)*149.5.

Device algorithm (no host-side sorting): per-axis linear interpolation is a
K=128 matmul with a "two-hot" hat-basis matrix e[g, t] = relu(1 - |pos_t - g|).
Grid 300 is split into 3 chunks of 128 lanes at stride 127; ALL three chunks
are computed for every point and accumulated in PSUM (the hat basis is zero
outside the containing chunk; duplicated boundary rows g=127 / g=254 are
zeroed in one of the two tables so each grid row contributes exactly once).

The dominant cost of this problem in this environment is the axon tunnel
(~55-80 MB/s each way, full-duplex), so the kernel minimizes bytes and
overlaps directions:
  - output is quantized on-device to int8 with a per-partition-row scale
    (q = out * 126.5/rowmax, |err| <= rowmax/126.5 < 1% of absmax << 2e-2);
    the f32 scales are bitcast into 4 extra int8 columns of the output
  - the f32->int8 second pass runs in the same program via a DRAM scratch
    tile (rowmax must be final before quantizing)
  - the exec path is a cached jax.jit(shard_map) around _bass_exec_p with
    output backing buffers created device-side and recycled via donation,
    so a warm call uploads only coords (24 MB) + tables (0.3 MB) and
    downloads int8 output (96 MB)
  - the call is split into S pipelined segment launches: segment s+1's
    upload/exec overlaps segment s's download (tunnel is full-duplex), and
    dequantization runs outside the fetch threads.

8 NeuronCores data-parallel over points; tables replicated.
"""

import os
import sys

sys.path.insert(0, "/opt/trn_rl_repo")
os.environ.setdefault("JAX_PLATFORMS", "axon,cpu")

import contextlib
import math
from concurrent.futures import ThreadPoolExecutor

import numpy as np

try:  # keep medium-sized host buffers on the heap so pages get reused
    import ctypes

    ctypes.CDLL("libc.so.6").mallopt(-3, 256 * 1024 * 1024)  # M_MMAP_THRESHOLD
except Exception:
    pass

import concourse.bass as bass
import concourse.mybir as mybir
from concourse import tile

F32 = mybir.dt.float32
I8 = mybir.dt.int8
AF = mybir.ActivationFunctionType
ALU = mybir.AluOpType

N_COMP = 48
G = 300
N_CORES = 8
TILE = 512
GROUP = 2 * TILE  # 1024 points per device group
SLAB = 8  # groups of coords per load slab
QCAP = 126.5  # quantization target range (<127 so saturation can't wrap)
PCHUNK = 4096  # pass-2 scratch columns per tile (multiple of TILE)
U16 = mybir.dt.uint16
POS_SCALE = 299.0 / 65535.0  # u16 fixed-point coord decode: pos = u * POS_SCALE


def _legalize_sync_waits(nc, max_waits=1):
    """This walrus build accepts at most one sync-wait per instruction; split
    extra waits onto preceding same-engine drains (same-queue => in order)."""
    n = 0
    for f in nc.m.functions:
        for bb in f.blocks:
            new_list = []
            for ins in bb.instructions:
                si = ins.sync_info
                waits = list(si.on_wait) if si and si.on_wait else []
                if len(waits) > max_waits:
                    head, tail = waits[:-max_waits], waits[-max_waits:]
                    for w in head:
                        n += 1
                        import bass_rust as _br
                        new_list.append(
                            _br.InstNoOp(
                                name=f"{ins.name}-wsplit-{n}",
                                engine=ins.engine,
                                ins=[],
                                outs=[],
                                sync_info=mybir.SyncInfo(on_wait=[w], on_update=[]),
                            )
                        )
                    ins.sync_info = mybir.SyncInfo(
                        on_wait=tail,
                        on_update=list(si.on_update) if si.on_update else [],
                    )
                new_list.append(ins)
            bb.instructions[:] = new_list
    return n


def _build_program(n_groups, num_devices=N_CORES):
    """Two-pass SPMD program for n_groups*GROUP points per core.

    Output tensor is [48, n_groups*GROUP + 4] int8: quantized values followed
    by 4 columns holding the bitcast f32 quantization multiplier per row.
    """
    npcp = n_groups * GROUP
    scratch_cols = n_groups * TILE  # packed halves: [128, 512] per group

    nc = bass.Bass("TRN2", target_bir_lowering=False, debug=False, num_devices=num_devices)
    d_coords = nc.dram_tensor("coords", [3, npcp], U16, kind="ExternalInput")
    d_lhsT = nc.dram_tensor("lhsT", [9, 128, 64], F32, kind="ExternalInput")
    d_bias = nc.dram_tensor("bias", [128, 3], F32, kind="ExternalInput")
    d_xmask = nc.dram_tensor("xmask", [128, PCHUNK], I8, kind="ExternalInput")
    d_out = nc.dram_tensor("out_q", [N_COMP, npcp + 4], I8, kind="ExternalOutput")

    with tile.TileContext(nc) as tc:
        with contextlib.ExitStack() as ctx:
            const = ctx.enter_context(tc.tile_pool(name="const", bufs=1))
            slabp = ctx.enter_context(tc.tile_pool(name="slabp", bufs=2))
            work = ctx.enter_context(tc.tile_pool(name="work", bufs=2))
            outp = ctx.enter_context(tc.tile_pool(name="outp", bufs=3))
            q2p = ctx.enter_context(tc.tile_pool(name="q2p", bufs=2))
            bcp = ctx.enter_context(tc.tile_pool(name="bcp", bufs=1, space="PSUM"))
            vpp = ctx.enter_context(tc.tile_pool(name="vpp", bufs=6, space="PSUM"))
            dramp = ctx.enter_context(tc.tile_pool(name="dramp", bufs=1, space="DRAM"))

            scratch = dramp.tile([128, scratch_cols], F32)

            lhsT = const.tile([128, 9 * 64], F32)
            nc.sync.dma_start(
                lhsT[:].rearrange("p (n d) -> p n d", d=64),
                d_lhsT.ap().rearrange("n p d -> p n d"),
            )
            biast = const.tile([128, 3], F32)
            nc.sync.dma_start(biast[:], d_bias.ap())
            xmask = const.tile([128, PCHUNK], I8)
            nc.sync.dma_start(xmask[:], d_xmask.ap())
            onest = const.tile([65, 128], F32)
            for a in range(3):
                nc.vector.memset(onest[32 * a : 32 * a + 1, :], 1.0)
            m = const.tile([128, 1], F32)
            nc.vector.memset(m[:], 1e-20)

            # ---- pass 1: interpolate, product, rowmax, f32 scratch ----
            for g in range(n_groups):
                s = g % SLAB
                if s == 0:
                    npts = min(SLAB * GROUP, npcp - g * GROUP)
                    slab_u = slabp.tile([65, SLAB * GROUP], U16, name="slab_u", tag="slab_u")
                    slab = slabp.tile([65, SLAB * GROUP], F32, name="slab", tag="slab")
                    for a in range(3):
                        nc.sync.dma_start(
                            slab_u[32 * a : 32 * a + 1, 0:npts],
                            d_coords.ap()[a : a + 1, g * GROUP : g * GROUP + npts],
                        )
                        nc.vector.tensor_copy(
                            slab[32 * a : 32 * a + 1, 0:npts],
                            slab_u[32 * a : 32 * a + 1, 0:npts],
                        )
                vps = []
                for a in range(3):
                    crow = slab[32 * a : 32 * a + 1, s * GROUP : (s + 1) * GROUP]
                    bc = bcp.tile([128, GROUP], F32, name=f"bc_{g}_{a}", tag="bc")
                    nc.tensor.matmul(
                        bc[:, 0:TILE], onest[32 * a : 32 * a + 1, :], crow[:, 0:TILE],
                        start=True, stop=True,
                    )
                    nc.tensor.matmul(
                        bc[:, TILE:GROUP], onest[32 * a : 32 * a + 1, :], crow[:, TILE:GROUP],
                        start=True, stop=True,
                    )
                    vp = vpp.tile([128, TILE], F32, name=f"vp_{g}_{a}", tag="vp")
                    for c in range(3):
                        tabs = work.tile([128, GROUP], F32, name=f"tabs_{g}_{a}_{c}", tag="tabs", bufs=3)
                        nc.scalar.activation(
                            tabs[:], bc[:], AF.Abs, bias=biast[:, c : c + 1], scale=POS_SCALE
                        )
                        eneg = work.tile([128, GROUP], F32, name=f"eneg_{g}_{a}_{c}", tag="eneg", bufs=3)
                        nc.vector.tensor_scalar(eneg[:], tabs[:], 1.0, 1.0, ALU.min, ALU.subtract)
                        lt = lhsT[:, (a * 3 + c) * 64 : (a * 3 + c + 1) * 64]
                        nc.tensor.matmul(
                            vp[0:64, :], lt, eneg[:, 0:TILE],
                            start=(c == 0), stop=(c == 2), tile_position=(0, 0),
                        )
                        nc.tensor.matmul(
                            vp[64:128, :], lt, eneg[:, TILE:GROUP],
                            start=(c == 0), stop=(c == 2), tile_position=(0, 64),
                        )
                    vps.append(vp)

                v1sb = outp.tile([128, TILE], F32, name=f"v1sb_{g}", tag="v1sb")
                nc.vector.tensor_copy(v1sb[:], vps[1][:])
                p01 = outp.tile([128, TILE], F32, name=f"p01_{g}", tag="p01")
                nc.vector.tensor_mul(p01[:], vps[0][:], v1sb[:])
                outt = outp.tile([128, TILE], F32, name=f"outt_{g}", tag="outt")
                nc.vector.tensor_mul(outt[:], vps[2][:], p01[:])

                mt = outp.tile([128, 1], F32, name=f"mt_{g}", tag="mt")
                nc.vector.tensor_reduce(
                    mt[:], outt[:], axis=mybir.AxisListType.X, op=ALU.max,
                    apply_absolute_value=True,
                )
                nc.vector.tensor_tensor(m[:], m[:], mt[:], op=ALU.max)

                nc.sync.dma_start(scratch[:, g * TILE : (g + 1) * TILE], outt[:])

            # ---- scales: rs = QCAP / max(row, row+64); bitcast into out ----
            mc = const.tile([128, 1], F32)
            nc.vector.memset(mc[:], 1e-20)
            nc.sync.dma_start(mc[0:N_COMP, :], m[64 : 64 + N_COMP, :])
            m2 = const.tile([128, 1], F32)
            nc.vector.memset(m2[:], 1.0)
            nc.vector.tensor_tensor(m2[0:N_COMP, :], m[0:N_COMP, :], mc[0:N_COMP, :], op=ALU.max)
            rs = const.tile([128, 1], F32)
            nc.vector.memset(rs[:], 1.0)
            rinv = const.tile([128, 1], F32)
            nc.vector.memset(rinv[:], 1.0)
            nc.vector.reciprocal(rinv[0:N_COMP, :], m2[0:N_COMP, :])
            nc.vector.tensor_scalar_mul(rs[0:N_COMP, :], rinv[0:N_COMP, :], QCAP)
            nc.sync.dma_start(rs[64 : 64 + N_COMP, :], rs[0:N_COMP, :])
            nc.sync.dma_start(
                d_out.ap()[:, npcp : npcp + 4], rs[0:N_COMP, :].bitcast(I8)
            )

            # ---- pass 2: quantize scratch -> int8 natural layout ----
            out_r = d_out.ap()[:, 0:npcp].rearrange("p (g h j) -> p g h j", h=2, j=TILE)
            n_chunks = math.ceil(scratch_cols / PCHUNK)
            for s in range(n_chunks):
                u0 = s * PCHUNK
                cols = min(PCHUNK, scratch_cols - u0)
                ngr = cols // TILE
                g0 = u0 // TILE
                qa = q2p.tile([128, PCHUNK], F32, name=f"qa_{s}", tag="qa")
                nc.sync.dma_start(qa[:, 0:cols], scratch[:, u0 : u0 + cols])
                qi = q2p.tile([128, PCHUNK], I8, name=f"qi_{s}", tag="qi")
                nc.vector.tensor_scalar_mul(qi[:, 0:cols], qa[:, 0:cols], rs[:, 0:1])
                # scramble so the tunnel's compressor takes its incompressible
                # fast path (host xors the mask back out)
                qx = q2p.tile([128, PCHUNK], I8, name=f"qx_{s}", tag="qx")
                nc.vector.tensor_tensor(
                    qx[:, 0:cols], qi[:, 0:cols], xmask[:, 0:cols], op=ALU.bitwise_xor
                )
                nc.sync.dma_start(
                    out_r[:, g0 : g0 + ngr, 0, :],
                    qx[0:N_COMP, 0:cols].rearrange("p (g j) -> p g j", j=TILE),
                )
                nc.sync.dma_start(
                    out_r[:, g0 : g0 + ngr, 1, :],
                    qx[64 : 64 + N_COMP, 0:cols].rearrange("p (g j) -> p g j", j=TILE),
                )

    from concourse.hw_specs import get_activation_tables
    import bass_rust as _br
    _br.insert_act_table_loads(nc, list(get_activation_tables(nc.m.arch).items()))
    _legalize_sync_waits(nc)
    return nc


# ---------------------------------------------------------------------------
# Cached PJRT exec path (modeled on concourse.bass2jax.run_bass_via_pjrt, but
# with a persistent jitted executable and donated, device-recycled output
# backing buffers so warm calls transfer no output-sized zeros).
# ---------------------------------------------------------------------------

_EXEC_CACHE: dict = {}
_HOST_BUFS: dict = {}  # n -> (out f32 [48,n], enc scratch f32 [3,n])

# fixed scramble mask; device xors output bytes with it, host xors back
_XMASK = np.random.default_rng(0xC0FFEE).integers(0, 256, (128, PCHUNK), dtype=np.uint8).astype(np.int8)
# host-side view over natural output columns: period 2*PCHUNK cols per chunk,
# emask[r, g*1024 + h*512 + j] = xmask[r + 64h, g*512 + j]
_EMASK = np.empty((N_COMP, 2 * PCHUNK), dtype=np.int8)
_EMASK.reshape(N_COMP, PCHUNK // TILE, 2, TILE)[:, :, 0, :] = _XMASK[:N_COMP].reshape(
    N_COMP, PCHUNK // TILE, TILE
)
_EMASK.reshape(N_COMP, PCHUNK // TILE, 2, TILE)[:, :, 1, :] = _XMASK[64 : 64 + N_COMP].reshape(
    N_COMP, PCHUNK // TILE, TILE
)


def _get_exec(seg_groups):
    key = seg_groups
    if key in _EXEC_CACHE:
        return _EXEC_CACHE[key]

    import jax
    import jax.numpy as jnp
    from jax.sharding import Mesh, PartitionSpec, NamedSharding
    try:
        from jax.experimental.shard_map import shard_map
    except ImportError:
        from jax.sharding import shard_map  # newer jax
    from concourse import bass2jax

    bass2jax.install_neuronx_cc_hook()

    nc = _build_program(seg_groups)
    partition_name = nc.partition_id_tensor.name if nc.partition_id_tensor else None

    in_names, out_names, out_avals = [], [], []
    for alloc in nc.m.functions[0].allocations:
        if not isinstance(alloc, mybir.MemoryLocationSet):
            continue
        name = alloc.memorylocations[0].name
        if alloc.kind == "ExternalInput":
            if name != partition_name:
                in_names.append(name)
        elif alloc.kind == "ExternalOutput":
            shape = tuple(alloc.tensor_shape)
            dtype = mybir.dt.np(alloc.dtype)
            out_names.append(name)
            out_avals.append(jax.core.ShapedArray(shape, dtype))
    n_params = len(in_names)
    n_outs = len(out_names)
    in_names = in_names + out_names
    if partition_name is not None:
        in_names.append(partition_name)

    dbg_names = []
    if nc.dbg_addr is not None:
        assert not nc.dbg_callbacks
        dbg_names = [nc.dbg_addr.name]

    def _body(*args):
        operands = list(args)
        if partition_name is not None:
            operands.append(bass2jax.partition_id_tensor())
        outs = bass2jax._bass_exec_p.bind(
            *operands,
            out_avals=tuple(out_avals),
            in_names=tuple(in_names),
            out_names=tuple(out_names),
            lowering_input_output_aliases=(),
            sim_require_finite=True,
            sim_require_nnan=True,
            nc=nc,
        )
        return tuple(outs)

    devices = jax.devices()[:N_CORES]
    assert len(devices) == N_CORES
    mesh = Mesh(np.asarray(devices), ("core",))
    sharding = NamedSharding(mesh, PartitionSpec("core"))
    repl_sharding = NamedSharding(mesh, PartitionSpec())
    # coords is per-core data; tables are replicated; backings are per-core
    replicated = {"lhsT", "bias", "xmask"}
    in_specs = tuple(
        PartitionSpec() if nm.split("/")[-1] in replicated else PartitionSpec("core")
        for nm in in_names[:n_params]
    ) + (PartitionSpec("core"),) * n_outs
    out_specs = (PartitionSpec("core"),) * n_outs
    donate = tuple(range(n_params, n_params + n_outs))
    fn = jax.jit(
        shard_map(_body, mesh=mesh, in_specs=in_specs, out_specs=out_specs, check_rep=False),
        donate_argnums=donate,
        keep_unused=True,
    )

    init_shapes = [
        (tuple([N_CORES * av.shape[0]] + list(av.shape[1:])), av.dtype) for av in out_avals
    ]
    init = jax.jit(
        lambda: tuple(jnp.zeros(s, d) for s, d in init_shapes),
        out_shardings=tuple(sharding for _ in init_shapes),
    )

    state = {
        "fn": fn,
        "init": init,
        "in_names": in_names[:n_params],
        "out_names": out_names,
        "backings": {},  # seg index -> tuple of backing arrays
        "sharding": sharding,
        "repl_sharding": repl_sharding,
        "devices": devices,
        "dbg_names": dbg_names,
    }
    _EXEC_CACHE[key] = state
    return state


def _schedule(n_groups):
    """Per-segment group counts: small first segments so download bytes start
    flowing early, then large segments for streaming efficiency."""
    env = os.environ.get("KSCHED")
    if env:
        sched = [int(x) for x in env.split(",")]
        assert sum(sched) == n_groups, (sched, n_groups)
        return sched
    if n_groups <= 12:
        return [n_groups]
    sched = []
    size, left = 8, n_groups
    while left > 2 * size and size < 128:
        sched.append(size)
        left -= size
        size *= 2
    while left > 0:
        take = min(128, left)
        sched.append(take)
        left -= take
    return sched


def kernel(xyz_sampled, param0, param1, param2):
    import jax
    import time as _time

    prof = bool(int(os.environ.get("KPROF", "0")))
    _t0 = _time.perf_counter()

    xyz = np.ascontiguousarray(xyz_sampled, dtype=np.float32)
    params = [
        np.ascontiguousarray(p.reshape(p.shape[1], p.shape[2]), dtype=np.float32)
        for p in (param0, param1, param2)
    ]
    n = xyz.shape[0]
    assert n % N_CORES == 0
    npc = n // N_CORES
    n_groups = math.ceil(npc / GROUP)
    sched = _schedule(n_groups)
    S = len(sched)

    sts = []  # (state, backing instance index) per segment
    seen: dict = {}
    for sg in sched:
        st = _get_exec(sg)
        inst = seen.get(sg, 0)
        seen[sg] = inst + 1
        if inst not in st["backings"]:
            st["backings"][inst] = list(st["init"]())
        sts.append((st, inst))
    st0 = sts[0][0]
    devices = st0["devices"]
    sharding = st0["sharding"]

    # --- host prep: u16 fixed-point coords; tables from params ---
    # u = floor((x+1)*32767.5 + 0.5), pos = u * (299/65535); |pos err| <= 0.00228
    if n not in _HOST_BUFS:
        _HOST_BUFS[n] = (
            np.empty((N_COMP, n), dtype=np.float32),
            np.empty((3, n), dtype=np.float32),
        )
    out, scr = _HOST_BUFS[n]
    np.multiply(xyz.T, 32767.5, out=scr)
    np.add(scr, 32768.0, out=scr)  # +32767.5 offset +0.5 for round-on-truncate
    xyzT = scr.astype(np.uint16)  # [3, n]
    lhsT9 = np.zeros((9, 128, 64), dtype=np.float32)
    for a in range(3):
        for c in range(3):
            seg = params[a][:, 127 * c : min(127 * c + 128, G)]
            lhsT9[a * 3 + c, : seg.shape[1], :N_COMP] = -seg.T
        lhsT9[a * 3 + 0, 127, :] = 0.0  # g=127 kept in chunk1 lane 0
        lhsT9[a * 3 + 1, 127, :] = 0.0  # g=254 kept in chunk2 lane 0
    bias = np.zeros((128, 3), dtype=np.float32)
    for c in range(3):
        bias[:, c] = -(127.0 * c + np.arange(128))

    lhsT_g = jax.device_put(lhsT9, st0["repl_sharding"])
    bias_g = jax.device_put(bias, st0["repl_sharding"])
    xmask_g = jax.device_put(_XMASK, st0["repl_sharding"])
    dbg_g = None
    if st0["dbg_names"]:
        dbg_g = jax.device_put(np.zeros((N_CORES, 2), np.uint32), sharding)

    # --- dispatch all segments (async) and queue D2H copies immediately;
    # the tunnel is full-duplex so segment s's download streams while
    # segment s+1 uploads/executes ---
    tasks = []
    g0 = 0
    for s in range(S):
        st, inst = sts[s]
        seg_npcp = sched[s] * GROUP
        c0 = g0 * GROUP
        cg = np.empty((N_CORES * 3, seg_npcp), dtype=np.uint16)
        cols = min(seg_npcp, npc - c0)  # last segment: real cols before padding
        for k in range(N_CORES):
            lo = k * npc + c0
            cg[3 * k : 3 * k + 3, :cols] = xyzT[:, lo : lo + cols]
            if cols < seg_npcp:
                cg[3 * k : 3 * k + 3, cols:] = cg[3 * k : 3 * k + 3, cols - 1 : cols]
        coords_g = jax.device_put(cg, sharding)
        args = []
        for name in st["in_names"]:
            base = name.split("/")[-1]
            if base == "coords":
                args.append(coords_g)
            elif base == "lhsT":
                args.append(lhsT_g)
            elif base == "bias":
                args.append(bias_g)
            elif base == "xmask":
                args.append(xmask_g)
            elif st["dbg_names"] and base == st["dbg_names"][0]:
                args.append(dbg_g)
            else:
                raise KeyError(f"unexpected program input {name}")
        outs = st["fn"](*args, *st["backings"][inst])
        st["backings"][inst] = list(outs)
        oq = outs[st["out_names"].index("out_q")]
        for sh in sorted(oq.addressable_shards, key=lambda x: x.index[0].start):
            sh.data.copy_to_host_async()
            tasks.append((c0, seg_npcp, sh.index[0].start // N_COMP, sh.data))
        g0 += sched[s]
    if prof:
        print(f"[kprof] {_time.perf_counter()-_t0:.3f} dispatched+copies queued")

    # --- collect all shards first (any Python work during streaming stalls
    # the axon receiver threads), then dequantize ---
    qks = []
    for c0, seg_npcp, k, shard in tasks:
        qks.append((c0, seg_npcp, k, np.asarray(shard)))
    if prof:
        print(f"[kprof] {_time.perf_counter()-_t0:.3f} collected")

    for c0, seg_npcp, k, qk in qks:
        cols = min(seg_npcp, npc - c0)
        scale = (
            1.0 / qk[:, seg_npcp : seg_npcp + 4].copy().view(np.float32).astype(np.float64)
        ).astype(np.float32)
        # un-scramble into scratch (np.asarray gives a read-only buffer)
        key = ("xor", seg_npcp)
        if key not in _HOST_BUFS:
            _HOST_BUFS[key] = np.empty((N_COMP, seg_npcp), dtype=np.int8)
        tmp = _HOST_BUFS[key]
        data = qk[:, :seg_npcp]
        period = 2 * PCHUNK
        nfull = seg_npcp // period
        if nfull:
            np.bitwise_xor(
                data[:, : nfull * period].reshape(N_COMP, nfull, period),
                _EMASK[:, None, :],
                out=tmp[:, : nfull * period].reshape(N_COMP, nfull, period),
            )
        rem = seg_npcp - nfull * period
        if rem:
            np.bitwise_xor(data[:, nfull * period :], _EMASK[:, :rem], out=tmp[:, nfull * period :])
        dst = out[:, k * npc + c0 : k * npc + c0 + cols]
        np.multiply(tmp[:, :cols], scale, out=dst, dtype=np.float32)
    if prof:
        print(f"[kprof] {_time.perf_counter()-_t0:.3f} done")
    return out


if __name__ == "__main__":
    rng = np.random.default_rng(0)
    n = int(os.environ.get("KN", 16 * 1024))
    xyz = rng.uniform(-1, 1, size=(n, 3)).astype(np.float32)
    ps = [0.2 * rng.standard_normal((1, N_COMP, G, 1)).astype(np.float32) for _ in range(3)]

    def ref_interp(p, coord):
        pp = p[0, :, :, 0]
        pos = (coord + 1.0) * 0.5 * (G - 1)
        i0 = np.clip(np.floor(pos).astype(np.int64), 0, G - 1)
        i1 = np.minimum(i0 + 1, G - 1)
        w = (pos - i0).astype(np.float32)
        return pp[:, i0] * (1.0 - w) + pp[:, i1] * w

    exp = ref_interp(ps[0], xyz[:, 0]) * ref_interp(ps[1], xyz[:, 1]) * ref_interp(ps[2], xyz[:, 2])
    got = kernel(xyz, *ps)
    err = np.abs(got - exp).max()
    print("max abs err:", err, "absmax:", np.abs(exp).max(), "rel:", err / np.abs(exp).max())
    import time
    for _ in range(2):
        t0 = time.perf_counter()
        kernel(xyz, *ps)
        print("warm wall:", time.perf_counter() - t0)
